# revision 1
# baseline (speedup 1.0000x reference)
"""Trainium2 Bass kernel for nn_MixedHeads (causal multi-head attention).

Reference computes, per (b, h):
  Q = x[b,:, :1024] @ Wq[h,:64,:1024].T      [T, 64]
  S = Q @ K.T * 0.125, causal mask, softmax
  O = P @ V, concat heads, pad to 2048 cols.

Sharding over 8 cores: core c -> batch b=c//2, heads h0=8*(c%2) .. h0+8.
Each core only reads its batch slice of x (8 MB) and its 8 heads' weights.

Device kernel (same SPMD program on all cores, data differs per core):
  phase 0/1 (fused): DMA x tiles [128,1024], PE-transpose to xsT [e,t] chunks,
     project with packed weights Wcat [1024, 1536] = [Q(8h*64) | K | V]:
       Q^T/K^T: out = Wchunk.T @ xsT  -> packed [128=2 heads x 64d, t]
       V:       out = xsT_tile.T @ Wv -> [t, 8h*64]  (natural PV layout)
     V stored with a ones column appended per head ([s, 65]) so the PV matmul
     also produces the softmax denominator as row 64.
  phase 2: per head, flash-style in S^T layout [s, tq]:
       S^T = K^T.T @ Q^T (per 128-s-chunk x 512-tq block, causal blocks only)
       P^T = exp(0.125*S^T + mask)   (mask only on 128-wide diagonal sub-block)
       acc[tq] += V'.T @ P^T         (PSUM accumulation over s chunks)
     acc [65, 512] -> DRAM; host divides rows 0..63 by row 64 and transposes.
"""

import sys

sys.path.insert(0, "/opt/trn_rl_repo")

import numpy as np

import concourse.bass as bass
import concourse.tile as tile
from concourse import bacc, mybir
from concourse.bass_utils import run_bass_kernel_spmd

F32 = mybir.dt.float32
F32R = mybir.dt.float32r
EXP = mybir.ActivationFunctionType.Exp


def _r(ap):
    return ap.bitcast(F32R)

B, TFULL, E, D = 4, 2048, 1024, 64
HPC = 8  # heads per core
NEG = -1.0e30
SCALE = 0.125


def build_nc(T=TFULL, reps=1):
    nq = T // 512   # tq chunks
    ns = T // 128   # s chunks
    nc = bacc.Bacc(None, target_bir_lowering=False, enable_partition_id=False)
    xbt = nc.dram_tensor("xbt", [E, T], F32, kind="ExternalInput")
    wcat = nc.dram_tensor("wcat", [E, 3 * HPC * D], F32, kind="ExternalInput")
    maskd = nc.dram_tensor("maskd", [128, 128], F32, kind="ExternalInput")
    o = nc.dram_tensor("o", [HPC, nq, 65, 512], F32, kind="ExternalOutput")

    with tile.TileContext(nc) as tc:
        with (
            tc.tile_pool(name="const", bufs=1) as constp,
            tc.tile_pool(name="qkstore", bufs=1) as qkp,
            tc.tile_pool(name="vstore", bufs=1) as vp,
        ):
            mask = constp.tile([128, 128], F32, tag="mask")
            nc.sync.dma_start(mask[:], maskd[:])
            Qs = qkp.tile([128, 4 * T], F32R, tag="qs")
            Ks = qkp.tile([128, 4 * T], F32R, tag="ks")
            Vs = vp.tile([128, ns * 520], F32R, tag="vs")
            nc.gpsimd.memset(Vs[:].bitcast(F32), 1.0)

            def emit_body():
                # ---------------- projection phase ----------------
                with (
                    tc.tile_pool(name="wpool", bufs=1) as wp,
                    tc.tile_pool(name="xsT", bufs=2) as xtp,
                    tc.tile_pool(name="prpsum", bufs=4, space="PSUM") as prp,
                ):
                    W = wp.tile([128, 8 * 1536], F32R, tag="w")
                    for e in range(8):
                        nc.sync.dma_start(
                            W[:, 1536 * e : 1536 * (e + 1)],
                            wcat[128 * e : 128 * (e + 1), :].bitcast(F32R),
                        )
                    for q in range(nq):
                        xsT = xtp.tile([128, 8 * 512], F32R, tag="xst")
                        nc.sync.dma_start(
                            xsT[:].rearrange("p (ec c) -> p ec c", ec=8),
                            xbt[:, 512 * q : 512 * (q + 1)]
                            .bitcast(F32R)
                            .rearrange("(ec p) c -> p ec c", p=128),
                        )
                        # Q^T / K^T groups: stationary = W cols, moving = xsT
                        for g in range(8):
                            pg = prp.tile([128, 512], F32, tag="pp")
                            for e in range(8):
                                nc.tensor.matmul(
                                    pg[:],
                                    W[:, 1536 * e + 128 * g : 1536 * e + 128 * (g + 1)],
                                    xsT[:, 512 * e : 512 * (e + 1)],
                                    start=(e == 0),
                                    stop=(e == 7),
                                )
                            dst = Qs if g < 4 else Ks
                            gg = g % 4
                            nc.scalar.copy(
                                dst[:, T * gg + 512 * q : T * gg + 512 * (q + 1)], pg[:]
                            )
                        # V: stationary = xsT tile, moving = W V-cols
                        for i in range(4):
                            pv = prp.tile([128, 512], F32, tag="pp")
                            for e in range(8):
                                nc.tensor.matmul(
                                    pv[:],
                                    xsT[:, 512 * e + 128 * i : 512 * e + 128 * (i + 1)],
                                    W[:, 1536 * e + 1024 : 1536 * e + 1536],
                                    start=(e == 0),
                                    stop=(e == 7),
                                )
                            c = 4 * q + i
                            nc.vector.tensor_copy(
                                Vs[:, 520 * c : 520 * c + 520].rearrange(
                                    "p (h d) -> p h d", h=8
                                )[:, :, 0:64],
                                pv[:].rearrange("p (h d) -> p h d", h=8),
                            )

                # ---------------- attention phase ----------------
                with (
                    tc.tile_pool(name="ppool", bufs=4) as ppool,
                    tc.tile_pool(name="ostage", bufs=2) as osp,
                    tc.tile_pool(name="spsum", bufs=2, space="PSUM") as spp,
                    tc.tile_pool(name="accpsum", bufs=4, space="PSUM") as accp,
                ):
                    for h in range(HPC):
                        row = 64 * (h % 2)
                        cb = T * (h // 2)
                        accs = [
                            accp.tile([128, 512], F32, tag="acc", name=f"acc{h}_{i}")
                            for i in range(nq)
                        ]
                        for j in range(ns):
                            i0 = j // 4
                            c0 = 128 * (j % 4)
                            pts = {}
                            # pair blocks two-at-a-time into [128,1024] psum
                            # tiles so one exp covers both (halves ACT count)
                            blocks = list(range(i0, nq))
                            for b0 in range(0, len(blocks), 2):
                                grp = blocks[b0 : b0 + 2]
                                w = 512 * len(grp)
                                spt = spp.tile([128, 1024], F32, tag="sp")
                                pt = ppool.tile([128, 1024], F32R, tag="pt")
                                lo = None
                                for k, i in enumerate(grp):
                                    n0 = c0 if i == i0 else 0
                                    if lo is None:
                                        lo = 512 * k + n0
                                    nc.tensor.matmul(
                                        spt[:, 512 * k + n0 : 512 * (k + 1)],
                                        Ks[
                                            row : row + 64,
                                            cb + 128 * j : cb + 128 * (j + 1),
                                        ],
                                        Qs[
                                            row : row + 64,
                                            cb + 512 * i + n0 : cb + 512 * (i + 1),
                                        ],
                                        start=True,
                                        stop=True,
                                    )
                                    pts[i] = (pt, 512 * k)
                                if grp[0] == i0:
                                    nc.vector.tensor_add(
                                        spt[:, c0 : c0 + 128],
                                        spt[:, c0 : c0 + 128],
                                        mask[:],
                                    )
                                nc.scalar.activation(
                                    pt[:, lo:w], spt[:, lo:w], EXP, scale=SCALE
                                )
                            for i in range(i0, nq):
                                pt, off = pts[i]
                                if i == i0 and c0 > 0:
                                    nc.gpsimd.memset(
                                        pt[:, off : off + c0].bitcast(F32), 0.0
                                    )
                                nc.tensor.matmul(
                                    accs[i][0:65, :],
                                    Vs[:, 520 * j + 65 * h : 520 * j + 65 * h + 65],
                                    pt[:, off : off + 512],
                                    start=(j == 0),
                                    stop=(j == 4 * i + 3),
                                )
                            if j % 4 == 3:
                                i = i0
                                ot = osp.tile([128, 512], F32, tag="ot")
                                nc.vector.tensor_copy(ot[0:65, :], accs[i][0:65, :])
                                nc.sync.dma_start(o[h, i], ot[0:65, :])

            for _rep in range(reps):
                emit_body()

    nc.compile()
    return nc


def make_in_maps(x, Wq, Wk, Wv, T=TFULL):
    x = np.asarray(x, np.float32)
    mask = np.where(
        np.arange(128)[None, :] >= np.arange(128)[:, None], 0.0, NEG
    ).astype(np.float32)
    in_maps = []
    for c in range(8):
        b, h0 = c // 2, HPC * (c % 2)
        xbv = np.ascontiguousarray(x[b, :T, :E].T)  # [E, T]
        parts = []
        for Wg in (Wq, Wk, Wv):
            wg = np.asarray(Wg, np.float32)[h0 : h0 + HPC, :D, :E]  # [8, 64, 1024]
            parts.append(wg.transpose(2, 0, 1).reshape(E, HPC * D))
        wcat = np.ascontiguousarray(np.concatenate(parts, axis=1))  # [1024, 1536]
        in_maps.append({"xbt": xbv, "wcat": wcat, "maskd": mask})
    return in_maps


def assemble(results, T=TFULL):
    out = np.zeros((B, TFULL, 2048), np.float32)
    for c in range(8):
        b, h0 = c // 2, HPC * (c % 2)
        ov = np.asarray(results[c]["o"])  # [8, nq, 65, 512]
        On = ov[:, :, :64, :] / ov[:, :, 64:65, :]  # [8, nq, 64, 512]
        blk = On.transpose(1, 3, 0, 2).reshape(T, HPC * D)  # [(i f), (h d)]
        out[b, :T, D * h0 : D * h0 + HPC * D] = blk
    return out


def kernel(**inputs):
    nc = build_nc()
    in_maps = make_in_maps(inputs["x"], inputs["Wq"], inputs["Wk"], inputs["Wv"])
    res = run_bass_kernel_spmd(nc, in_maps, core_ids=list(range(8)))
    return assemble(res.results)

